# revision 1
# baseline (speedup 1.0000x reference)
"""Trainium2 Bass kernel for nn_ClassConfusionLoss.

Self-contained: takes FULL inputs pred (64,64,128,128) f32, gt (64,64,128,128) i32,
shards the spatial W axis across 8 NeuronCores, computes per-core partial weighted
covariance M (64x64, as a 128x128 PSUM block pair), reduces on host and applies the
final row-normalization + trace (O(C^2), negligible).

Math: the reference's global scalars num_pos and S = sum(n*w_raw) scale cov by
alpha = num_pos/S, which cancels in cov / cov.sum(axis=1). So only
M[c,k] = sum_p n_p*w_raw_p*x_pc*x_pk is needed, where x[b,c,w,h] =
pred[b,c,w,h]/(sum_c' pred[c,c',w,h] + eps)  (batch index c — valid since B == C),
n = sum_c(gt==1), w_raw = 1 + exp(-ent), ent = -sum_c x*log(x+eps).

Per core (w-slab of 16, processed as 8 adjacent-w pairs):
  pred_nat/gt_nat [(j*64+c)=128p, b=64, h=128] bf16   <- SWDGE cast DMA
  D/N/E[h, j, b] via lhsT-trick matmuls (chunk lhsT = nat[:, b, :], rhs = half-indicator)
  Rt[(j*64+b), h] = PE-transpose of 1/D ; x_nat = pred_nat * Rt (partition-bcast c==b)
  L = Ln(x+1e-12) (ACT), XL = x*L, E = sum_c XL, m = (exp(E)+1)*N
  xT_b = PE-transpose of x_nat[:, b, :] ; Y_b = xT_b * m-bcast
  M_ps[128,128] += Y_b^T @ xT_b  (512 accumulating matmuls)
Host: M = sum_cores(M_ps[0:64,0:64] + M_ps[64:128,64:128]); cov /= cov.sum(1);
loss = (cov.sum() - trace)/C.
"""

import numpy as np

B, C, W, H = 64, 64, 128, 128
NCORES = 8
WS = W // NCORES          # 16 w's per core
NPAIR = WS // 2           # 8 w-pairs per core
EPS = 1e-12

_CACHE = {}


def _build_nc():
    from contextlib import ExitStack

    import concourse.bass as bass
    import concourse.tile as tile
    from concourse import bacc, masks, mybir

    F32 = mybir.dt.float32
    BF16 = mybir.dt.bfloat16
    I32 = mybir.dt.int32
    AF = mybir.ActivationFunctionType
    OP = mybir.AluOpType

    nc = bacc.Bacc("TRN2", target_bir_lowering=False, debug=False)

    pred_t = nc.dram_tensor("pred", [B, C, WS, H], F32, kind="ExternalInput")
    gt_t = nc.dram_tensor("gt", [B, C, WS, H], I32, kind="ExternalInput")
    mout_t = nc.dram_tensor("m_out", [128, 128], F32, kind="ExternalOutput")

    # DRAM strides (elements) of the shard tensor (B, C, WS, H)
    SB_, SC_, SW_, SH_ = C * WS * H, WS * H, H, 1

    with tile.TileContext(nc) as tc, ExitStack() as ctx:
        singles = ctx.enter_context(tc.tile_pool(name="singles", bufs=1))
        pred_pool = ctx.enter_context(tc.tile_pool(name="pred", bufs=2))
        gt_pool = ctx.enter_context(tc.tile_pool(name="gt", bufs=2))
        x_pool = ctx.enter_context(tc.tile_pool(name="x", bufs=2))
        l_pool = ctx.enter_context(tc.tile_pool(name="l", bufs=2))
        xl_pool = ctx.enter_context(tc.tile_pool(name="xl", bufs=2))
        sm_pool = ctx.enter_context(tc.tile_pool(name="sm", bufs=3))
        yx_pool = ctx.enter_context(tc.tile_pool(name="yx", bufs=4))
        ps_dn = ctx.enter_context(tc.tile_pool(name="ps_dn", bufs=2, space="PSUM"))
        ps_er = ctx.enter_context(tc.tile_pool(name="ps_er", bufs=2, space="PSUM"))
        ps_xt = ctx.enter_context(tc.tile_pool(name="ps_xt", bufs=3, space="PSUM"))
        ps_m = ctx.enter_context(tc.tile_pool(name="ps_m", bufs=1, space="PSUM"))

        ident_b = singles.tile([128, 128], BF16)
        masks.make_identity(nc, ident_b[:])
        ident_f = singles.tile([128, 128], F32)
        masks.make_identity(nc, ident_f[:])
        ind = singles.tile([128, 2], BF16)
        nc.vector.memset(ind[:], 0.0)
        nc.vector.memset(ind[0:64, 0:1], 1.0)
        nc.vector.memset(ind[64:128, 1:2], 1.0)
        eps_t = singles.tile([128, 1], F32)
        nc.vector.memset(eps_t[:], EPS)

        m_ps = ps_m.tile([128, 128], F32)
        first_mm = [True]

        for wp in range(NPAIR):
            # ---- loads: [(j*64+c)=128p, b=64, h=128], w-major partitions ----
            def nat_in(t, j):
                return bass.AP(
                    tensor=t.ap().tensor,
                    offset=(wp * 2 + j) * SW_,
                    ap=[[SC_, 64], [SB_, 64], [SH_, H]],
                )

            pn = pred_pool.tile([128, 64, H], BF16)
            gn = gt_pool.tile([128, 64, H], BF16)
            for j in range(2):
                nc.gpsimd.dma_start(out=pn[j * 64:(j + 1) * 64], in_=nat_in(pred_t, j))
                nc.gpsimd.dma_start(out=gn[j * 64:(j + 1) * 64], in_=nat_in(gt_t, j))

            # ---- D / N via lhsT-trick ----
            dn = ps_dn.tile([128, 256], F32)
            Dv = dn[:, 0:128].rearrange("p (j b) -> p j b", j=2)
            Nv = dn[:, 128:256].rearrange("p (j b) -> p j b", j=2)
            for b in range(64):
                nc.tensor.matmul(Dv[:, :, b], pn[:, b, :], ind[:],
                                 start=True, stop=True, skip_group_check=True)
            for b in range(64):
                nc.tensor.matmul(Nv[:, :, b], gn[:, b, :], ind[:],
                                 start=True, stop=True, skip_group_check=True)

            # ---- R = 1/D, transpose to [(j*64+b), h], evac to bf16 ----
            rp = sm_pool.tile([128, 128], F32, tag="rp")
            nc.vector.reciprocal(rp[:], dn[:, 0:128])
            er = ps_er.tile([128, 256], F32)
            Ev = er[:, 0:128].rearrange("p (j b) -> p j b", j=2)
            rt_ps = er[:, 128:256]
            nc.tensor.matmul(rt_ps, rp[:], ident_f[:], is_transpose=True,
                             start=True, stop=True, skip_group_check=True)
            rt = sm_pool.tile([128, 128], BF16, tag="rt")
            nc.scalar.copy(rt[:], rt_ps)

            # ---- x = pred * Rt  (partition p=(j,c) reads Rt row (j,c): batch c) ----
            x = x_pool.tile([128, 64, H], BF16)
            rt_b = bass.AP(tensor=rt.tensor, offset=rt.offset,
                           ap=[rt.ap[0], [0, 64], [1, H]])
            nc.vector.tensor_mul(x[:], pn[:], rt_b)

            # ---- L = ln(x + eps); XL = x * L ----
            L = l_pool.tile([128, 64, H], BF16)
            nc.scalar.activation(L[:], x[:], AF.Ln, bias=eps_t[:], scale=1.0)
            xl = xl_pool.tile([128, 64, H], BF16)
            nc.vector.tensor_mul(xl[:], x[:], L[:])

            # ---- E = sum_c XL (lhsT-trick) ----
            for b in range(64):
                nc.tensor.matmul(Ev[:, :, b], xl[:, b, :], ind[:],
                                 start=True, stop=True, skip_group_check=True)

            # ---- m = (exp(E) + 1) * N ----
            expe = sm_pool.tile([128, 128], BF16, tag="expe")
            nc.scalar.activation(expe[:], er[:, 0:128], AF.Exp, bias=0.0, scale=1.0)
            mm = sm_pool.tile([128, 128], BF16, tag="m")
            nc.vector.scalar_tensor_tensor(
                out=mm[:], in0=expe[:], scalar=1.0, in1=dn[:, 128:256],
                op0=OP.add, op1=OP.mult,
            )

            # ---- transposes, Y, main MMs (spans of 8 b's) ----
            for sp in range(8):
                xt_ps = ps_xt.tile([128, 1024], BF16)
                for k in range(8):
                    b = sp * 8 + k
                    nc.tensor.matmul(xt_ps[:, k * 128:(k + 1) * 128], x[:, b, :],
                                     ident_b[:], is_transpose=True,
                                     start=True, stop=True, skip_group_check=True)
                y_sb = yx_pool.tile([128, 1024], BF16, tag="y")
                m_b = bass.AP(tensor=mm.tensor, offset=mm.offset + sp * 8,
                              ap=[mm.ap[0], [1, 8], [64, 2], [0, 64]])
                nc.vector.tensor_mul(y_sb[:], xt_ps[:], m_b)
                xt_sb = yx_pool.tile([128, 1024], BF16, tag="xt")
                nc.vector.tensor_copy(xt_sb[:], xt_ps[:])
                for k in range(8):
                    nc.tensor.matmul(
                        m_ps[:], y_sb[:, k * 128:(k + 1) * 128],
                        xt_sb[:, k * 128:(k + 1) * 128],
                        start=first_mm[0], stop=(wp == NPAIR - 1 and sp == 7 and k == 7),
                        skip_group_check=True,
                    )
                    first_mm[0] = False

        m_sb = singles.tile([128, 128], F32)
        nc.vector.tensor_copy(m_sb[:], m_ps[:])
        nc.sync.dma_start(out=mout_t.ap(), in_=m_sb[:])

    nc.compile()
    return nc


def _get_nc():
    if "nc" not in _CACHE:
        _CACHE["nc"] = _build_nc()
    return _CACHE["nc"]


def kernel(pred: np.ndarray, gt: np.ndarray) -> np.ndarray:
    from concourse.bass_utils import run_bass_kernel_spmd

    pred = np.ascontiguousarray(pred, dtype=np.float32)
    gt = np.ascontiguousarray(gt, dtype=np.int32)
    nc = _get_nc()

    in_maps = []
    for s in range(NCORES):
        in_maps.append({
            "pred": np.ascontiguousarray(pred[:, :, s * WS:(s + 1) * WS, :]),
            "gt": np.ascontiguousarray(gt[:, :, s * WS:(s + 1) * WS, :]),
        })
    res = run_bass_kernel_spmd(nc, in_maps, core_ids=list(range(NCORES)))

    M = np.zeros((64, 64), dtype=np.float32)
    for r in res.results:
        mo = r["m_out"]
        M += mo[0:64, 0:64] + mo[64:128, 64:128]
    cov = M / M.sum(axis=1)
    return np.float32((cov.sum() - np.trace(cov)) / C)



# revision 2
# speedup vs baseline: 1.1216x; 1.1216x over previous
"""Trainium2 Bass kernel for nn_ClassConfusionLoss.

Self-contained: takes FULL inputs pred (64,64,128,128) f32, gt (64,64,128,128) i32,
shards the spatial W axis across 8 NeuronCores, computes per-core partial weighted
covariance M (64x64, as a 128x128 PSUM block pair), reduces on host and applies the
final row-normalization + trace (O(C^2), negligible).

Math: the reference's global scalars num_pos and S scale cov by alpha = num_pos/S,
which cancels in cov / cov.sum(axis=1). So only
M[c,k] = sum_p n_p*w_raw_p*x_pc*x_pk is needed, where x[b,c,w,h] =
pred[b,c,w,h]/(sum_c' pred[c,c',w,h] + eps)  (batch index c -- valid since B == C),
n = sum_c(gt==1), w_raw = 1 + exp(ent') with ent' = sum_c x*log(x+eps).

Layout per core (w-slab of 16, processed as 8 adjacent-w pairs):
  partition p = q*64 + c with q = b&1; free dims [t=b>>1 (32), j (2), h (128)].
  This makes each DMA descriptor span a contiguous (w,w+1)x(h) 256-element run
  in DRAM (1KB src / 512B dst), which keeps the cast DMA at full bus rate.

Per w-pair:
  pnx/gnx [128p, 32, 2, 128] bf16 <- 4 SWDGE cast DMAs (q halves x {pred, gt})
  D/N[h, j*64+b] via per-(q,t,j) matmuls vs ones (contraction over c)
  rp = 1/D; rt2[(q,c), (j,h)] = PE-transpose of rp (both q halves), bf16
  x = pnx * rt2-broadcast;  L = ln(x+eps);  xl = x*L
  E[h, j*64+b] via per-(q,t,j) matmuls of xl vs ones
  m = (exp(E)+1)*N;  sqm = exp(0.5*ln(m))      (single act table set: ln/exp/copy)
  per span of 8 b's: 16 PE transposes of x -> xt_ps[128,1024];
    z = xt_ps * sqm-broadcast  (z = sqrt(m) * x^T);  M_ps += z^T z per 128-col block
Host: M = sum_cores(M_ps[0:64,0:64] + M_ps[64:128,64:128]); cov /= cov.sum(1);
loss = (cov.sum() - trace)/C.
"""

import numpy as np

B, C, W, H = 64, 64, 128, 128
NCORES = 8
WS = W // NCORES          # 16 w's per core
NPAIR = WS // 2           # 8 w-pairs per core
EPS = 1e-12

# spans (of 64 total: wp*8+sp) whose z-scale op runs on gpsimd instead of DVE
Z_POOL_SPANS = frozenset()

_CACHE = {}


def _build_nc():
    from contextlib import ExitStack

    import concourse.bass as bass
    import concourse.tile as tile
    from concourse import bacc, masks, mybir
    from concourse.hw_specs import get_activation_tables

    F32 = mybir.dt.float32
    BF16 = mybir.dt.bfloat16
    I32 = mybir.dt.int32
    AF = mybir.ActivationFunctionType
    OP = mybir.AluOpType

    nc = bacc.Bacc("TRN2", target_bir_lowering=False, debug=False)

    pred_t = nc.dram_tensor("pred", [B, C, WS, H], F32, kind="ExternalInput")
    gt_t = nc.dram_tensor("gt", [B, C, WS, H], I32, kind="ExternalInput")
    mout_t = nc.dram_tensor("m_out", [128, 128], F32, kind="ExternalOutput")

    # DRAM strides (elements) of the shard tensor (B, C, WS, H)
    SB_, SC_, SW_, SH_ = C * WS * H, WS * H, H, 1

    with tile.TileContext(nc) as tc, ExitStack() as ctx:
        singles = ctx.enter_context(tc.tile_pool(name="singles", bufs=1))
        pred_pool = ctx.enter_context(tc.tile_pool(name="pred", bufs=2))
        gt_pool = ctx.enter_context(tc.tile_pool(name="gt", bufs=2))
        x_pool = ctx.enter_context(tc.tile_pool(name="x", bufs=2))
        l_pool = ctx.enter_context(tc.tile_pool(name="l", bufs=2))
        xl_pool = ctx.enter_context(tc.tile_pool(name="xl", bufs=2))
        sm_pool = ctx.enter_context(tc.tile_pool(name="sm", bufs=2))
        z_pool = ctx.enter_context(tc.tile_pool(name="z", bufs=4))
        ps_dn = ctx.enter_context(tc.tile_pool(name="ps_dn", bufs=2, space="PSUM"))
        ps_er = ctx.enter_context(tc.tile_pool(name="ps_er", bufs=2, space="PSUM"))
        ps_xt = ctx.enter_context(tc.tile_pool(name="ps_xt", bufs=3, space="PSUM"))
        ps_m = ctx.enter_context(tc.tile_pool(name="ps_m", bufs=1, space="PSUM"))

        ident_b = singles.tile([128, 128], BF16)
        masks.make_identity(nc, ident_b[:])
        ident_f = singles.tile([128, 128], F32)
        masks.make_identity(nc, ident_f[:])
        ones_c = singles.tile([128, 1], BF16)
        nc.vector.memset(ones_c[:], 1.0)
        eps_t = singles.tile([128, 1], F32)
        nc.vector.memset(eps_t[:], EPS)
        zero_t = singles.tile([128, 1], F32)
        nc.vector.memset(zero_t[:], 0.0)

        # Pin the ln+exp+copy activation table once so the compiler pass does
        # not insert a reload at every ln<->exp switch.
        tabs = get_activation_tables(nc.m.arch)
        set_id = next(
            i for i, s in enumerate(tabs.values())
            if AF.Ln in s and AF.Exp in s and AF.Copy in s
        )
        load_inst = mybir.InstLoadActFuncSet(
            name=nc.get_next_instruction_name(), act_func_set_id=set_id,
            ins=[], outs=[],
        )
        load_inst.engine = mybir.EngineType.Activation
        nc.scalar.add_instruction(load_inst)

        m_ps = ps_m.tile([128, 128], F32)
        first_mm = [True]

        for wp in range(NPAIR):
            # ---- loads: [(q*64+c)p, t=32, j=2, h=128]; src runs = 256 f32 ----
            pnx = pred_pool.tile([128, 32, 2, 128], BF16)
            gnx = gt_pool.tile([128, 32, 2, 128], BF16)
            for q in range(2):
                off = wp * 2 * SW_ + q * SB_
                nc.gpsimd.dma_start(
                    out=pnx[q * 64:(q + 1) * 64],
                    in_=bass.AP(tensor=pred_t.ap().tensor, offset=off,
                                ap=[[SC_, 64], [2 * SB_, 32], [1, 256]]),
                )
                nc.gpsimd.dma_start(
                    out=gnx[q * 64:(q + 1) * 64],
                    in_=bass.AP(tensor=gt_t.ap().tensor, offset=off,
                                ap=[[SC_, 64], [2 * SB_, 32], [1, 256]]),
                )

            # ---- D/N[h, j*64+b] = sum_c pnx/gnx over the q-half partitions ----
            dn = ps_dn.tile([128, 256], F32)
            for q in range(2):
                on = ones_c[q * 64:(q + 1) * 64, :]
                for t in range(32):
                    b = 2 * t + q
                    for j in range(2):
                        col = j * 64 + b
                        nc.tensor.matmul(dn[:, col:col + 1],
                                         pnx[q * 64:(q + 1) * 64, t, j, :], on,
                                         start=True, stop=True,
                                         skip_group_check=True)
                        nc.tensor.matmul(dn[:, 128 + col:129 + col],
                                         gnx[q * 64:(q + 1) * 64, t, j, :], on,
                                         start=True, stop=True,
                                         skip_group_check=True)

            # ---- rp = 1/D; rt2[(q,c), (j,h)] = rp[h, j*64+c] (both q) ----
            rp = sm_pool.tile([128, 128], F32, tag="rp")
            nc.vector.reciprocal(rp[:], dn[:, 0:128])
            er = ps_er.tile([128, 384], F32)
            rt_ps = er[:, 128:384].rearrange("p (j h) -> p j h", j=2)
            for q in range(2):
                for j in range(2):
                    nc.tensor.matmul(rt_ps[q * 64:(q + 1) * 64, j],
                                     rp[:, j * 64:(j + 1) * 64], ident_f[:],
                                     is_transpose=True, start=True, stop=True,
                                     skip_group_check=True)
            rt2 = sm_pool.tile([128, 256], BF16, tag="rt2")
            nc.scalar.copy(rt2[:], er[:, 128:384])

            # ---- x = pnx * rt2-broadcast (rt2 free (j,h), bcast over t) ----
            x = x_pool.tile([128, 32, 2, 128], BF16)
            rt_b = bass.AP(tensor=rt2.tensor, offset=rt2.offset,
                           ap=[rt2.ap[0], [0, 32], [128, 2], [1, 128]])
            nc.vector.tensor_mul(x[:], pnx[:], rt_b)

            # ---- L = ln(x + eps); xl = x * L ----
            L = l_pool.tile([128, 32, 2, 128], BF16)
            nc.scalar.activation(L[:], x[:], AF.Ln, bias=eps_t[:], scale=1.0)
            xl = xl_pool.tile([128, 32, 2, 128], BF16)
            nc.vector.tensor_mul(xl[:], x[:], L[:])

            # ---- E[h, j*64+b] = sum_c xl ----
            for q in range(2):
                on = ones_c[q * 64:(q + 1) * 64, :]
                for t in range(32):
                    b = 2 * t + q
                    for j in range(2):
                        col = j * 64 + b
                        nc.tensor.matmul(er[:, col:col + 1],
                                         xl[q * 64:(q + 1) * 64, t, j, :], on,
                                         start=True, stop=True,
                                         skip_group_check=True)

            # ---- sqm = sqrt((exp(E)+1)*N) = exp(0.5*ln(m)) ----
            expe = sm_pool.tile([128, 128], BF16, tag="expe")
            nc.scalar.activation(expe[:], er[:, 0:128], AF.Exp,
                                 bias=zero_t[:], scale=1.0)
            m32 = sm_pool.tile([128, 128], F32, tag="m32")
            nc.vector.scalar_tensor_tensor(
                out=m32[:], in0=expe[:], scalar=1.0, in1=dn[:, 128:256],
                op0=OP.add, op1=OP.mult,
            )
            lnm = sm_pool.tile([128, 128], F32, tag="lnm")
            nc.scalar.activation(lnm[:], m32[:], AF.Ln, bias=eps_t[:], scale=1.0)
            sqm = sm_pool.tile([128, 128], BF16, tag="sqm")
            nc.scalar.activation(sqm[:], lnm[:], AF.Exp, bias=zero_t[:], scale=0.5)

            # ---- spans: transposes, z = xt * sqm, main MMs ----
            for sp in range(8):
                xt_ps = ps_xt.tile([128, 1024], BF16)
                for k in range(8):
                    b = sp * 8 + k
                    q, t = b & 1, b >> 1
                    qs = slice(q * 64, (q + 1) * 64)
                    for j in range(2):
                        nc.tensor.matmul(
                            xt_ps[:, k * 128 + j * 64:k * 128 + (j + 1) * 64],
                            x[qs, t, j, :], ident_b[qs, qs],
                            is_transpose=True, start=True, stop=True,
                            skip_group_check=True)
                z = z_pool.tile([128, 1024], BF16)
                sq_b = bass.AP(tensor=sqm.tensor, offset=sqm.offset + sp * 8,
                               ap=[sqm.ap[0], [1, 8], [64, 2], [0, 64]])
                eng = nc.gpsimd if (wp * 8 + sp) in Z_POOL_SPANS else nc.vector
                eng.tensor_mul(z[:], xt_ps[:], sq_b)
                for k in range(8):
                    nc.tensor.matmul(
                        m_ps[:], z[:, k * 128:(k + 1) * 128],
                        z[:, k * 128:(k + 1) * 128],
                        start=first_mm[0],
                        stop=(wp == NPAIR - 1 and sp == 7 and k == 7),
                        skip_group_check=True,
                    )
                    first_mm[0] = False

        m_sb = singles.tile([128, 128], F32)
        nc.vector.tensor_copy(m_sb[:], m_ps[:])
        nc.sync.dma_start(out=mout_t.ap(), in_=m_sb[:])

    nc.compile()
    return nc


def _get_nc():
    if "nc" not in _CACHE:
        _CACHE["nc"] = _build_nc()
    return _CACHE["nc"]


def kernel(pred: np.ndarray, gt: np.ndarray) -> np.ndarray:
    from concourse.bass_utils import run_bass_kernel_spmd

    pred = np.ascontiguousarray(pred, dtype=np.float32)
    gt = np.ascontiguousarray(gt, dtype=np.int32)
    nc = _get_nc()

    in_maps = []
    for s in range(NCORES):
        in_maps.append({
            "pred": np.ascontiguousarray(pred[:, :, s * WS:(s + 1) * WS, :]),
            "gt": np.ascontiguousarray(gt[:, :, s * WS:(s + 1) * WS, :]),
        })
    res = run_bass_kernel_spmd(nc, in_maps, core_ids=list(range(NCORES)))

    M = np.zeros((64, 64), dtype=np.float32)
    for r in res.results:
        mo = r["m_out"]
        M += mo[0:64, 0:64] + mo[64:128, 64:128]
    cov = M / M.sum(axis=1)
    return np.float32((cov.sum() - np.trace(cov)) / C)
